# revision 19
# baseline (speedup 1.0000x reference)
"""Trainium2 Bass kernel for nn_BinaryMNModel (binary Markov-network clique scoring).

Math: for each batch row b,
    ll[b] = sum_c sum_j f[c,j] * prod_s ( bc[j,s] ? x[b,vars[c,s]] : 1-x[b,vars[c,s]] )

Centered multilinear basis: with b_s = x[b,vars[c,s]] - 0.5 in [-0.5, 0.5],
    score[c,b] = h0[c] + sum_s h_s[c] b_s + sum_{s<t} h_st[c] b_s b_t
               + h_012[c] b0 b1 b2
(h = an 8x8 +-0.5^k transform of the 8 factor entries, exact on host in f64).
Centering shrinks |b| and the products, so bf16 rounding error stays ~1e-3
of the final sum.  Summing over cliques:
  - h0 becomes one host-side scalar,
  - linear terms fold into a V-length weight vector w (host scatter-add),
    so sum_c(linear) = b @ w  (on-device f32 matvec, V-sharded),
  - only the 4 quad/cubic monomials need the gathered values (bf16).

Sharding: cliques are sharded across the 8 cores (2500 each); the b@w matvec
is V-sharded.  Each core returns a partial [256] vector; host sums them.

Per core on device:
  - dma_gather (GPSIMD SWDGE) pulls bf16 rows of bt=(x^T-0.5) [V, B] from
    DRAM, one merged gather per chunk-group covering all 3 clique slots
    (fewer gpsimd instructions -> no DMASW sem-lane-reuse drains, and the
    4 SWDGE queues = 4 Q7 core pairs overlap descriptor generation, which
    at ~9 ns/row is the pacing resource).
  - DVE computes the 4 products p01, p02, p12, p012=p01*a2 per group in
    bf16 (2x_1P mode: 2 elem/cycle).
  - PE reduces each weighted monomial over cliques via bf16 matmuls with
    the h column as stationary [128,1], in the PE's 128x32 column-groups
    0/32/64 (col-group 3 is a known TRN2 HW bug), accumulating into psum
    rows 0/32/64; the f32 b@w matvec rides the row-0 chain.  The three
    rows are summed on DVE at the end.
  - idx is loaded in two slices so the first gathers start as soon as the
    first ~37KB lands instead of waiting for the full index load.
"""

import os

import numpy as np

# ---------------------------------------------------------------- constants
B = 256
V = 5000
C = 20000
S = 3
NCOMB = 8
N_CORES = 8

C_SHARD = C // N_CORES          # 2500 cliques per core
CHUNKS = 20                     # 2560 = 20 * 128
C_PAD = CHUNKS * 128            # padded cliques per core

# group sizes (in 128-clique chunks): ramped so the first products start
# early while later groups are large enough to keep all 4 queues busy
GROUP_CHUNKS = [int(t) for t in os.environ.get("K_GROUPS", "2,2,2,2,2,2,2,2,1,1,1,1").split(",")]
assert sum(GROUP_CHUNKS) == CHUNKS
N_GROUPS = len(GROUP_CHUNKS)
# leading groups whose idx is loaded in the first (small) idx DMA slice
IDX_SPLIT = int(os.environ.get("K_IDX_SPLIT", "4"))

V_SHARD = V // N_CORES          # 625
V_CHUNKS = 5                    # padded to 640 = 5 * 128

IDX_COLS_TOTAL = 3 * C_PAD // 16   # 480

# aux32 layout (f32): [xv V_CHUNKS*B | wv V_CHUNKS]
WV_OFF = V_CHUNKS * B
AUX32_COLS = WV_OFF + V_CHUNKS
AUX16_COLS = 4 * CHUNKS

N_QUEUES = int(os.environ.get("K_NQ", "4"))
MERGE_SLOTS = os.environ.get("K_MERGE", "1") == "1"   # one gather per group (vs per slot)
SCALAR_DMA = os.environ.get("K_SCALAR_DMA", "1") == "1"  # aux32 on ACT HWDGE ring
SINGLE_PACKET = os.environ.get("K_SP", "1") == "1"
DUMMY_GATHER = os.environ.get("K_DUMMY", "0") == "1"  # warm-up gather before the real ones
PSUM_OUT = os.environ.get("K_PSUM_OUT", "1") == "1"   # DMA psum rows; host sums

_PROGRAM = None  # compiled program cache: (nc, out_name)


def _group_offsets():
    c_off = [0]
    for gc in GROUP_CHUNKS:
        c_off.append(c_off[-1] + gc)
    return c_off


def _build_program():
    import concourse.bass as bass
    import concourse.mybir as mybir
    from concourse import bacc, tile

    f32 = mybir.dt.float32
    bf16 = mybir.dt.bfloat16
    i16 = mybir.dt.int16
    MULT = mybir.AluOpType.mult

    nc = bacc.Bacc(
        "TRN2",
        target_bir_lowering=False,
        debug=False,
        enable_asserts=False,
        num_devices=N_CORES,
        num_swdge_queues=max(N_QUEUES, 1),
    )

    xt_d = nc.dram_tensor("xt", [V, B], bf16, kind="ExternalInput")
    idx_d = nc.dram_tensor("idx", [128, IDX_COLS_TOTAL], i16, kind="ExternalInput")
    a16_d = nc.dram_tensor("aux16", [128, AUX16_COLS], bf16, kind="ExternalInput")
    a32_d = nc.dram_tensor("aux32", [128, AUX32_COLS], f32, kind="ExternalInput")
    out_rows = 65 if PSUM_OUT else 1
    out_d = nc.dram_tensor("out", [out_rows, B], f32, kind="ExternalOutput")

    c_off = _group_offsets()
    # idx column offset of each group block (each group: 3*gc*8 cols)
    idx_off = [0]
    for gc in GROUP_CHUNKS:
        idx_off.append(idx_off[-1] + 3 * gc * 8)
    split_col = idx_off[min(IDX_SPLIT, N_GROUPS)]

    with tile.TileContext(nc) as tc:
        with (
            tc.tile_pool(name="persist", bufs=1) as pp,
            tc.tile_pool(name="prod", bufs=8) as prodp,
            tc.tile_pool(name="ps", bufs=1, space="PSUM") as psp,
        ):
            idx_a = pp.tile([128, max(split_col, 8)], i16, tag="idxa")
            idx_b = pp.tile(
                [128, max(IDX_COLS_TOTAL - split_col, 8)], i16, tag="idxb"
            )

            def idx_slice(lo, hi):
                if hi <= split_col:
                    return idx_a[:, lo:hi]
                assert lo >= split_col
                return idx_b[:, lo - split_col : hi - split_col]
            a16_t = pp.tile([128, AUX16_COLS], bf16, tag="aux16")
            a32_t = pp.tile([128, AUX32_COLS], f32, tag="aux32")
            a_all = pp.tile([128, 3 * CHUNKS, B], bf16, tag="a_all", name="a_all")
            out_sb = pp.tile([1, B], f32, tag="out_sb")
            tmp_s = pp.tile([1, B], f32, tag="tmp_s")
            tmp_t = pp.tile([1, B], f32, tag="tmp_t")
            psum_t = psp.tile([128, B], f32, tag="psum")

            if os.environ.get("K_LOADLIB", "0") == "1":
                from concourse import library_config as _lc

                nc.gpsimd.load_library(_lc.mlp)
            if DUMMY_GATHER:
                # tiny warm-up gather: triggers the Q7 ucode load/warm path
                # before the real idx data even arrives (reads row 0 of xt)
                dum_i = pp.tile([128, 8], i16, tag="dum_i")
                dum_o = pp.tile([128, 1, B], bf16, tag="dum_o")
                nc.gpsimd.memset(dum_i[:], 0)
                nc.gpsimd.dma_gather(
                    dum_o[:], xt_d[:], dum_i[:], 128, nc.gpsimd.to_reg(128), B,
                    queue_num=0, single_packet=SINGLE_PACKET,
                )

            # input loads: first idx slice gates the first gathers; aux32
            # rides the scalar-engine HWDGE ring so it doesn't queue behind
            # the idx slices on the sync ring.
            if split_col > 0:
                nc.sync.dma_start(idx_a[:], idx_d[:, :split_col])
            if split_col < IDX_COLS_TOTAL:
                nc.sync.dma_start(idx_b[:], idx_d[:, split_col:])
            aux_eng = nc.scalar if SCALAR_DMA else nc.sync
            aux_eng.dma_start(a16_t[:], a16_d[:])
            aux_eng.dma_start(a32_t[:], a32_d[:])

            # all gathers first: they are gpsimd's only work and pace the
            # kernel.  One merged gather per group (3 slots concatenated);
            # queue assignment must be pure round-robin: Tile's 8 DMASW sem
            # lanes are assigned round-robin per call and each lane is locked
            # to one SWDGE queue.  num_idxs registers are hoisted: a MOVE on
            # the Pool SEQ costs ~0.5us, so emit one per distinct size, not
            # one per call.
            size_regs = {}
            sizes = set()
            for gc in GROUP_CHUNKS:
                sizes.add(3 * gc * 128 if MERGE_SLOTS else gc * 128)
            for n in sorted(sizes):
                size_regs[n] = nc.gpsimd.to_reg(n)
            gi = 0
            for g, gc in enumerate(GROUP_CHUNKS):
                c0 = c_off[g]
                if MERGE_SLOTS:
                    n_idx = 3 * gc * 128
                    nc.gpsimd.dma_gather(
                        a_all[:, 3 * c0 : 3 * c0 + 3 * gc, :],
                        xt_d[:],
                        idx_slice(idx_off[g], idx_off[g + 1]),
                        n_idx,
                        size_regs[n_idx],
                        B,
                        queue_num=(gi + DUMMY_GATHER) % N_QUEUES,
                        single_packet=SINGLE_PACKET,
                    )
                    gi += 1
                else:
                    for s in range(S):
                        n_idx = gc * 128
                        nc.gpsimd.dma_gather(
                            a_all[:, 3 * c0 + s * gc : 3 * c0 + (s + 1) * gc, :],
                            xt_d[:],
                            idx_slice(
                                idx_off[g] + s * gc * 8,
                                idx_off[g] + (s + 1) * gc * 8,
                            ),
                            n_idx,
                            size_regs[n_idx],
                            B,
                            queue_num=(gi + DUMMY_GATHER) % N_QUEUES,
                            single_packet=SINGLE_PACKET,
                        )
                        gi += 1

            # PE accumulation: weighted clique reductions run in the PE's
            # 128x32 column-groups (col-group 3 is a known HW bug, so only
            # groups 0/32/64 are used).  m01 -> row 0, m02 -> row 32,
            # m12 -> row 64; m012 is split between rows 32 and 64.
            row_started = set()

            def mm(mono, chunk, moving_ap, last=False):
                coef_col = mono * CHUNKS + chunk
                lhs = a16_t[:, coef_col : coef_col + 1]
                if mono < 3:
                    row = 32 * mono
                else:
                    row = 32 if chunk < CHUNKS // 2 else 64
                nc.tensor.matmul(
                    psum_t[row : row + 1, :],
                    lhs,
                    moving_ap,
                    start=(row not in row_started),
                    stop=last,
                    tile_position=(0, row),
                )
                row_started.add(row)

            def matvec():
                # linear terms: b @ w in f32 on the PE row-0 chain
                for j in range(V_CHUNKS):
                    nc.tensor.matmul(
                        psum_t[0:1, :],
                        a32_t[:, WV_OFF + j : WV_OFF + j + 1],
                        a32_t[:, j * B : (j + 1) * B],
                        start=(0 not in row_started),
                        stop=False,
                        tile_position=(0, 0),
                    )
                row_started.add(0)

            for g, gc in enumerate(GROUP_CHUNKS):
                c0 = c_off[g]
                base = 3 * c0
                a0g = a_all[:, base : base + gc, :]
                a1g = a_all[:, base + gc : base + 2 * gc, :]
                a2g = a_all[:, base + 2 * gc : base + 3 * gc, :]
                p01 = prodp.tile([128, gc, B], bf16, tag="p01", name="p01")
                p02 = prodp.tile([128, gc, B], bf16, tag="p02", name="p02")
                p12 = prodp.tile([128, gc, B], bf16, tag="p12", name="p12")
                p012 = prodp.tile([128, gc, B], bf16, tag="p012", name="p012")
                nc.vector.tensor_tensor(p01[:], a0g, a1g, MULT)
                nc.vector.tensor_tensor(p02[:], a0g, a2g, MULT)
                nc.vector.tensor_tensor(p12[:], a1g, a2g, MULT)
                nc.vector.tensor_tensor(p012[:], p01[:], a2g, MULT)
                last = g == N_GROUPS - 1
                for c in range(gc):
                    ci = c0 + c
                    lc = last and c == gc - 1
                    mm(0, ci, p01[:, c, :], last=lc)   # row 0 ends with m01
                    mm(1, ci, p02[:, c, :], last=lc)   # row 32 ends with m02
                    mm(2, ci, p12[:, c, :])
                    mm(3, ci, p012[:, c, :], last=lc)  # row 64 ends with m012
                if g == min(2, N_GROUPS - 1):
                    # matvec mid-stream: aux32 has landed by now, and the
                    # fp32 LOW/HIGH pair must not open the row-0 chain
                    matvec()

            if PSUM_OUT:
                # one wide copy of psum partitions 0..64 to SBUF, then DMA;
                # the host sums rows 0/32/64 (the three accumulation chains)
                out65 = pp.tile([65, B], f32, tag="out65")
                nc.vector.tensor_copy(out65[:], psum_t[0:65, :])
                nc.sync.dma_start(out_d[:], out65[:])
            else:
                # combine the 3 chain rows (0, 32, 64) into the output
                # (DVE may read at most one PSUM operand per instruction)
                nc.vector.tensor_copy(tmp_s[:], psum_t[0:1, :])
                nc.vector.tensor_add(tmp_t[:], tmp_s[:], psum_t[32:33, :])
                nc.vector.tensor_add(out_sb[:], tmp_t[:], psum_t[64:65, :])
                nc.sync.dma_start(out_d[:], out_sb[:])

    nc.compile()
    return nc, out_d.name


def get_program():
    global _PROGRAM
    if _PROGRAM is None:
        _PROGRAM = _build_program()
    return _PROGRAM


# ---------------------------------------------------------------- host prep
def _monomial_transform_centered(all_factors: np.ndarray) -> np.ndarray:
    """h[c,T] such that score[c,b] = sum_T h[c,T] * prod_{s in T} b_s,
    with b_s = a_s - 0.5 and T a bitmask over slots (bit s = slot s)."""
    M = np.zeros((NCOMB, NCOMB), dtype=np.float64)
    for T in range(NCOMB):
        nT = bin(T).count("1")
        for j in range(NCOMB):
            prod = 0.5 ** (S - nT)
            for s in range(S):
                if (T >> s) & 1:
                    prod *= 1.0 if (j >> (S - 1 - s)) & 1 else -1.0
            M[T, j] = prod
    return all_factors.astype(np.float64) @ M.T


def _wrap_idx(idx: np.ndarray) -> np.ndarray:
    """[N] int -> [128, N//16] int16 dma_gather layout (idx i at
    partition i%16, col i//16, replicated across the 8 q7 cores)."""
    w = idx.reshape(-1, 16).T.astype(np.int16)  # [16, N//16]
    return np.tile(w, (8, 1))


def _chunk_layout(v: np.ndarray) -> np.ndarray:
    """[C_PAD] -> [128, CHUNKS]: element i at partition i%128, col i//128."""
    return np.ascontiguousarray(v.reshape(CHUNKS, 128).T)


def prepare_inputs(x, all_vars, all_factors):
    import ml_dtypes

    bfloat16 = ml_dtypes.bfloat16
    x = np.asarray(x, dtype=np.float32)
    all_vars = np.asarray(all_vars)
    all_factors = np.asarray(all_factors, dtype=np.float32)

    bt32 = np.ascontiguousarray(x.T) - np.float32(0.5)  # [V, B] f32 centered
    bt = bt32.astype(bfloat16)                          # [V, B] bf16

    h = _monomial_transform_centered(all_factors)  # [C, 8] f64
    T01, T02, T12, T012 = 0b011, 0b101, 0b110, 0b111

    const0 = float(h[:, 0].sum())
    w = np.zeros(V, dtype=np.float64)
    for s in range(S):
        np.add.at(w, all_vars[:, s], h[:, 1 << s])
    w = w.astype(np.float32)
    h32 = h.astype(np.float32)

    c_off = _group_offsets()
    in_maps = []
    for k in range(N_CORES):
        sl = slice(k * C_SHARD, (k + 1) * C_SHARD)
        pad = C_PAD - C_SHARD

        # padded per-core vars: [C_PAD, S]; clique ci at (p=ci%128, chunk=ci//128)
        vp = np.concatenate(
            [all_vars[sl], np.zeros((pad, S), all_vars.dtype)], axis=0
        )
        idx_blocks = []
        for g, gc in enumerate(GROUP_CHUNKS):
            c0 = c_off[g]
            # row i = s*(gc*128) + c_local*128 + p -> var of clique
            # (c0+c_local)*128 + p, slot s
            blk = np.concatenate(
                [
                    vp[c0 * 128 : (c0 + gc) * 128, s]
                    for s in range(S)
                ]
            )
            idx_blocks.append(_wrap_idx(blk))
        idx_arr = np.ascontiguousarray(np.concatenate(idx_blocks, axis=1))
        assert idx_arr.shape == (128, IDX_COLS_TOTAL)

        coef_cols = []
        for t in (T01, T02, T12, T012):
            hh = np.concatenate([h32[sl, t], np.zeros(pad, np.float32)])
            coef_cols.append(_chunk_layout(hh))
        a16 = np.ascontiguousarray(
            np.concatenate(coef_cols, axis=1).astype(bfloat16)
        )
        assert a16.shape == (128, AUX16_COLS)

        vs = slice(k * V_SHARD, (k + 1) * V_SHARD)
        vpad = V_CHUNKS * 128 - V_SHARD
        xv = np.concatenate([bt32[vs], np.zeros((vpad, B), np.float32)])
        xv = xv.reshape(V_CHUNKS, 128, B).transpose(1, 0, 2).reshape(128, V_CHUNKS * B)
        wv = np.concatenate([w[vs], np.zeros(vpad, np.float32)])
        wv = np.ascontiguousarray(wv.reshape(V_CHUNKS, 128).T)
        a32 = np.ascontiguousarray(
            np.concatenate([xv, wv], axis=1, dtype=np.float32)
        )
        assert a32.shape == (128, AUX32_COLS)

        in_maps.append({"xt": bt, "idx": idx_arr, "aux16": a16, "aux32": a32})

    return in_maps, const0


# ---------------------------------------------------------------- entry
def run(inputs: dict, trace: bool = False):
    from concourse import bass_utils

    in_maps, const0 = prepare_inputs(
        inputs["x"], inputs["all_vars"], inputs["all_factors"]
    )
    nc, out_name = get_program()
    res = bass_utils.run_bass_kernel_spmd(
        nc, in_maps, core_ids=list(range(N_CORES)), trace=trace
    )
    rows = (0, 32, 64) if PSUM_OUT else (0,)
    partials = np.stack(
        [
            np.asarray(r[out_name]).reshape(-1, B)[list(rows)].sum(axis=0)
            for r in res.results
        ]
    )
    ll = partials.astype(np.float64).sum(axis=0) + const0
    return ll.astype(np.float32), res


def kernel(x, binary_combinations, all_vars, all_factors):
    out, _ = run(
        {"x": x, "all_vars": all_vars, "all_factors": all_factors}
    )
    return out


# revision 22
# speedup vs baseline: 1.2176x; 1.2176x over previous
"""Trainium2 Bass kernel for nn_BinaryMNModel (binary Markov-network clique scoring).

Math: for each batch row b,
    ll[b] = sum_c sum_j f[c,j] * prod_s ( bc[j,s] ? x[b,vars[c,s]] : 1-x[b,vars[c,s]] )

Centered multilinear basis: with b_s = x[b,vars[c,s]] - 0.5 in [-0.5, 0.5],
    score[c,b] = h0[c] + sum_s h_s[c] b_s + sum_{s<t} h_st[c] b_s b_t
               + h_012[c] b0 b1 b2
(h = an 8x8 +-0.5^k transform of the 8 factor entries, exact on host in f64).
Centering shrinks |b| and the products, so bf16 rounding error stays ~1e-3
of the final sum.  Summing over cliques:
  - h0 becomes one host-side scalar,
  - linear terms fold into a V-length weight vector w (host scatter-add),
    so sum_c(linear) = b @ w  (on-device f32 matvec, V-sharded),
  - only the 4 quad/cubic monomials need the gathered values (bf16).

Sharding: cliques are sharded across the 8 cores (2500 each); the b@w matvec
is V-sharded.  Each core returns a partial [256] vector; host sums them.

Per core on device:
  - dma_gather (GPSIMD SWDGE) pulls bf16 rows of bt=(x^T-0.5) [V, B] from
    DRAM, one merged gather per chunk-group covering all 3 clique slots
    (fewer gpsimd instructions -> no DMASW sem-lane-reuse drains, and the
    4 SWDGE queues = 4 Q7 core pairs overlap descriptor generation, which
    at ~9 ns/row is the pacing resource).
  - DVE computes the 4 products p01, p02, p12, p012=p01*a2 per group in
    bf16 (2x_1P mode: 2 elem/cycle).
  - PE reduces each weighted monomial over cliques via bf16 matmuls with
    the h column as stationary [128,1], in the PE's 128x32 column-groups
    0/32/64 (col-group 3 is a known TRN2 HW bug), accumulating into psum
    rows 0/32/64; the f32 b@w matvec rides the row-0 chain.  The three
    rows are summed on DVE at the end.
  - idx is loaded in two slices so the first gathers start as soon as the
    first ~37KB lands instead of waiting for the full index load.
"""

import os

import numpy as np

# ---------------------------------------------------------------- constants
B = 256
V = 5000
C = 20000
S = 3
NCOMB = 8
N_CORES = 8

C_SHARD = C // N_CORES          # 2500 cliques per core
CHUNKS = 20                     # 2560 = 20 * 128
C_PAD = CHUNKS * 128            # padded cliques per core

# group sizes (in 128-clique chunks): ramped so the first products start
# early while later groups are large enough to keep all 4 queues busy
GROUP_CHUNKS = [int(t) for t in os.environ.get("K_GROUPS", "2,2,2,2,2,2,2,2,1,1,1,1").split(",")]
assert sum(GROUP_CHUNKS) == CHUNKS
N_GROUPS = len(GROUP_CHUNKS)
# leading groups whose idx is loaded in the first (small) idx DMA slice
IDX_SPLIT = int(os.environ.get("K_IDX_SPLIT", "4"))

V_SHARD = V // N_CORES          # 625
V_CHUNKS = 5                    # padded to 640 = 5 * 128

IDX_COLS_TOTAL = 3 * C_PAD // 16   # 480

# aux32 layout (f32): [xv V_CHUNKS*B | wv V_CHUNKS]
WV_OFF = V_CHUNKS * B
AUX32_COLS = WV_OFF + V_CHUNKS
AUX16_COLS = 4 * CHUNKS

N_QUEUES = int(os.environ.get("K_NQ", "4"))
MERGE_SLOTS = os.environ.get("K_MERGE", "1") == "1"   # one gather per group (vs per slot)
SCALAR_DMA = os.environ.get("K_SCALAR_DMA", "1") == "1"  # aux32 on ACT HWDGE ring
SINGLE_PACKET = os.environ.get("K_SP", "1") == "1"
DUMMY_GATHER = os.environ.get("K_DUMMY", "0") == "1"  # warm-up gather before the real ones
PSUM_OUT = os.environ.get("K_PSUM_OUT", "0") == "1"   # DMA psum rows; host sums

_PROGRAM = None  # compiled program cache: (nc, out_name)


def _group_offsets():
    c_off = [0]
    for gc in GROUP_CHUNKS:
        c_off.append(c_off[-1] + gc)
    return c_off


def _build_program():
    import concourse.bass as bass
    import concourse.mybir as mybir
    from concourse import bacc, tile

    f32 = mybir.dt.float32
    bf16 = mybir.dt.bfloat16
    i16 = mybir.dt.int16
    MULT = mybir.AluOpType.mult

    nc = bacc.Bacc(
        "TRN2",
        target_bir_lowering=False,
        debug=False,
        enable_asserts=False,
        num_devices=N_CORES,
        num_swdge_queues=max(N_QUEUES, 1),
    )

    xt_d = nc.dram_tensor("xt", [V, B], bf16, kind="ExternalInput")
    idx_d = nc.dram_tensor("idx", [128, IDX_COLS_TOTAL], i16, kind="ExternalInput")
    a16_d = nc.dram_tensor("aux16", [128, AUX16_COLS], bf16, kind="ExternalInput")
    a32_d = nc.dram_tensor("aux32", [128, AUX32_COLS], f32, kind="ExternalInput")
    out_rows = 65 if PSUM_OUT else 1
    out_d = nc.dram_tensor("out", [out_rows, B], f32, kind="ExternalOutput")

    c_off = _group_offsets()
    # idx column offset of each group block (each group: 3*gc*8 cols)
    idx_off = [0]
    for gc in GROUP_CHUNKS:
        idx_off.append(idx_off[-1] + 3 * gc * 8)
    split_col = idx_off[min(IDX_SPLIT, N_GROUPS)]

    with tile.TileContext(nc) as tc:
        with (
            tc.tile_pool(name="persist", bufs=1) as pp,
            tc.tile_pool(name="prod", bufs=4) as prodp,
            tc.tile_pool(name="ps", bufs=1, space="PSUM") as psp,
        ):
            idx_a = pp.tile([128, max(split_col, 8)], i16, tag="idxa")
            idx_b = pp.tile(
                [128, max(IDX_COLS_TOTAL - split_col, 8)], i16, tag="idxb"
            )

            def idx_slice(lo, hi):
                if hi <= split_col:
                    return idx_a[:, lo:hi]
                assert lo >= split_col
                return idx_b[:, lo - split_col : hi - split_col]
            a16_t = pp.tile([128, AUX16_COLS], bf16, tag="aux16")
            a32_t = pp.tile([128, AUX32_COLS], f32, tag="aux32")
            a_all = pp.tile([128, 3 * CHUNKS, B], bf16, tag="a_all", name="a_all")
            out_sb = pp.tile([1, B], f32, tag="out_sb")
            tmp_s = pp.tile([1, B], f32, tag="tmp_s")
            tmp_t = pp.tile([1, B], f32, tag="tmp_t")
            psum_t = psp.tile([128, B], f32, tag="psum")

            if os.environ.get("K_LOADLIB", "0") == "1":
                from concourse import library_config as _lc

                nc.gpsimd.load_library(_lc.mlp)
            if DUMMY_GATHER:
                # tiny warm-up gather: triggers the Q7 ucode load/warm path
                # before the real idx data even arrives (reads row 0 of xt)
                dum_i = pp.tile([128, 8], i16, tag="dum_i")
                dum_o = pp.tile([128, 1, B], bf16, tag="dum_o")
                nc.gpsimd.memset(dum_i[:], 0)
                nc.gpsimd.dma_gather(
                    dum_o[:], xt_d[:], dum_i[:], 128, nc.gpsimd.to_reg(128), B,
                    queue_num=0, single_packet=SINGLE_PACKET,
                )

            # input loads: first idx slice gates the first gathers; aux32
            # rides the scalar-engine HWDGE ring so it doesn't queue behind
            # the idx slices on the sync ring.
            if split_col > 0:
                nc.sync.dma_start(idx_a[:], idx_d[:, :split_col])
            if split_col < IDX_COLS_TOTAL:
                nc.sync.dma_start(idx_b[:], idx_d[:, split_col:])
            nc.sync.dma_start(a16_t[:], a16_d[:])
            aux_eng = nc.scalar if SCALAR_DMA else nc.sync
            aux_eng.dma_start(a32_t[:], a32_d[:])

            # all gathers first: they are gpsimd's only work and pace the
            # kernel.  One merged gather per group (3 slots concatenated);
            # queue assignment must be pure round-robin: Tile's 8 DMASW sem
            # lanes are assigned round-robin per call and each lane is locked
            # to one SWDGE queue.  num_idxs registers are hoisted: a MOVE on
            # the Pool SEQ costs ~0.5us, so emit one per distinct size, not
            # one per call.
            size_regs = {}
            sizes = set()
            for gc in GROUP_CHUNKS:
                sizes.add(3 * gc * 128 if MERGE_SLOTS else gc * 128)
            for n in sorted(sizes):
                size_regs[n] = nc.gpsimd.to_reg(n)
            gi = 0
            for g, gc in enumerate(GROUP_CHUNKS):
                c0 = c_off[g]
                if MERGE_SLOTS:
                    n_idx = 3 * gc * 128
                    nc.gpsimd.dma_gather(
                        a_all[:, 3 * c0 : 3 * c0 + 3 * gc, :],
                        xt_d[:],
                        idx_slice(idx_off[g], idx_off[g + 1]),
                        n_idx,
                        size_regs[n_idx],
                        B,
                        queue_num=(gi + DUMMY_GATHER) % N_QUEUES,
                        single_packet=SINGLE_PACKET,
                    )
                    gi += 1
                else:
                    for s in range(S):
                        n_idx = gc * 128
                        nc.gpsimd.dma_gather(
                            a_all[:, 3 * c0 + s * gc : 3 * c0 + (s + 1) * gc, :],
                            xt_d[:],
                            idx_slice(
                                idx_off[g] + s * gc * 8,
                                idx_off[g] + (s + 1) * gc * 8,
                            ),
                            n_idx,
                            size_regs[n_idx],
                            B,
                            queue_num=(gi + DUMMY_GATHER) % N_QUEUES,
                            single_packet=SINGLE_PACKET,
                        )
                        gi += 1

            # PE accumulation: weighted clique reductions run in the PE's
            # 128x32 column-groups (col-group 3 is a known HW bug, so only
            # groups 0/32/64 are used).  m01 -> row 0, m02 -> row 32,
            # m12 -> row 64; m012 is split between rows 32 and 64.
            row_started = set()

            def mm(mono, chunk, moving_ap, last=False):
                coef_col = mono * CHUNKS + chunk
                lhs = a16_t[:, coef_col : coef_col + 1]
                if mono < 3:
                    row = 32 * mono
                else:
                    row = 32 if chunk < CHUNKS // 2 else 64
                nc.tensor.matmul(
                    psum_t[row : row + 1, :],
                    lhs,
                    moving_ap,
                    start=(row not in row_started),
                    stop=last,
                    tile_position=(0, row),
                )
                row_started.add(row)

            def matvec():
                # linear terms: b @ w in f32 on the PE row-0 chain
                for j in range(V_CHUNKS):
                    nc.tensor.matmul(
                        psum_t[0:1, :],
                        a32_t[:, WV_OFF + j : WV_OFF + j + 1],
                        a32_t[:, j * B : (j + 1) * B],
                        start=(0 not in row_started),
                        stop=False,
                        tile_position=(0, 0),
                    )
                row_started.add(0)

            for g, gc in enumerate(GROUP_CHUNKS):
                c0 = c_off[g]
                base = 3 * c0
                a0g = a_all[:, base : base + gc, :]
                a1g = a_all[:, base + gc : base + 2 * gc, :]
                a2g = a_all[:, base + 2 * gc : base + 3 * gc, :]
                p01 = prodp.tile([128, gc, B], bf16, tag="p01", name="p01")
                p02 = prodp.tile([128, gc, B], bf16, tag="p02", name="p02")
                p12 = prodp.tile([128, gc, B], bf16, tag="p12", name="p12")
                p012 = prodp.tile([128, gc, B], bf16, tag="p012", name="p012")
                nc.vector.tensor_tensor(p01[:], a0g, a1g, MULT)
                nc.vector.tensor_tensor(p02[:], a0g, a2g, MULT)
                nc.vector.tensor_tensor(p12[:], a1g, a2g, MULT)
                nc.vector.tensor_tensor(p012[:], p01[:], a2g, MULT)
                last = g == N_GROUPS - 1
                for c in range(gc):
                    ci = c0 + c
                    lc = last and c == gc - 1
                    mm(0, ci, p01[:, c, :], last=lc)   # row 0 ends with m01
                    mm(1, ci, p02[:, c, :], last=lc)   # row 32 ends with m02
                    mm(2, ci, p12[:, c, :])
                    mm(3, ci, p012[:, c, :], last=lc)  # row 64 ends with m012
                if g == min(2, N_GROUPS - 1):
                    # matvec mid-stream: aux32 has landed by now, and the
                    # fp32 LOW/HIGH pair must not open the row-0 chain
                    matvec()

            if PSUM_OUT:
                # one wide copy of psum partitions 0..64 to SBUF, then DMA;
                # the host sums rows 0/32/64 (the three accumulation chains)
                out65 = pp.tile([65, B], f32, tag="out65")
                nc.vector.tensor_copy(out65[:], psum_t[0:65, :])
                nc.sync.dma_start(out_d[:], out65[:])
            else:
                # combine the 3 chain rows (0, 32, 64) into the output
                # (DVE may read at most one PSUM operand per instruction)
                nc.vector.tensor_copy(tmp_s[:], psum_t[0:1, :])
                nc.vector.tensor_add(tmp_t[:], tmp_s[:], psum_t[32:33, :])
                nc.vector.tensor_add(out_sb[:], tmp_t[:], psum_t[64:65, :])
                nc.sync.dma_start(out_d[:], out_sb[:])

    nc.compile()
    return nc, out_d.name


def get_program():
    global _PROGRAM
    if _PROGRAM is None:
        _PROGRAM = _build_program()
    return _PROGRAM


# ---------------------------------------------------------------- host prep
def _monomial_transform_centered(all_factors: np.ndarray) -> np.ndarray:
    """h[c,T] such that score[c,b] = sum_T h[c,T] * prod_{s in T} b_s,
    with b_s = a_s - 0.5 and T a bitmask over slots (bit s = slot s)."""
    M = np.zeros((NCOMB, NCOMB), dtype=np.float64)
    for T in range(NCOMB):
        nT = bin(T).count("1")
        for j in range(NCOMB):
            prod = 0.5 ** (S - nT)
            for s in range(S):
                if (T >> s) & 1:
                    prod *= 1.0 if (j >> (S - 1 - s)) & 1 else -1.0
            M[T, j] = prod
    return all_factors.astype(np.float64) @ M.T


def _wrap_idx(idx: np.ndarray) -> np.ndarray:
    """[N] int -> [128, N//16] int16 dma_gather layout (idx i at
    partition i%16, col i//16, replicated across the 8 q7 cores)."""
    w = idx.reshape(-1, 16).T.astype(np.int16)  # [16, N//16]
    return np.tile(w, (8, 1))


def _chunk_layout(v: np.ndarray) -> np.ndarray:
    """[C_PAD] -> [128, CHUNKS]: element i at partition i%128, col i//128."""
    return np.ascontiguousarray(v.reshape(CHUNKS, 128).T)


def prepare_inputs(x, all_vars, all_factors):
    import ml_dtypes

    bfloat16 = ml_dtypes.bfloat16
    x = np.asarray(x, dtype=np.float32)
    all_vars = np.asarray(all_vars)
    all_factors = np.asarray(all_factors, dtype=np.float32)

    bt32 = np.ascontiguousarray(x.T) - np.float32(0.5)  # [V, B] f32 centered
    bt = bt32.astype(bfloat16)                          # [V, B] bf16

    h = _monomial_transform_centered(all_factors)  # [C, 8] f64
    T01, T02, T12, T012 = 0b011, 0b101, 0b110, 0b111

    const0 = float(h[:, 0].sum())
    w = np.zeros(V, dtype=np.float64)
    for s in range(S):
        np.add.at(w, all_vars[:, s], h[:, 1 << s])
    w = w.astype(np.float32)
    h32 = h.astype(np.float32)

    c_off = _group_offsets()
    in_maps = []
    for k in range(N_CORES):
        sl = slice(k * C_SHARD, (k + 1) * C_SHARD)
        pad = C_PAD - C_SHARD

        # padded per-core vars: [C_PAD, S]; clique ci at (p=ci%128, chunk=ci//128)
        vp = np.concatenate(
            [all_vars[sl], np.zeros((pad, S), all_vars.dtype)], axis=0
        )
        idx_blocks = []
        for g, gc in enumerate(GROUP_CHUNKS):
            c0 = c_off[g]
            # row i = s*(gc*128) + c_local*128 + p -> var of clique
            # (c0+c_local)*128 + p, slot s
            blk = np.concatenate(
                [
                    vp[c0 * 128 : (c0 + gc) * 128, s]
                    for s in range(S)
                ]
            )
            idx_blocks.append(_wrap_idx(blk))
        idx_arr = np.ascontiguousarray(np.concatenate(idx_blocks, axis=1))
        assert idx_arr.shape == (128, IDX_COLS_TOTAL)

        coef_cols = []
        for t in (T01, T02, T12, T012):
            hh = np.concatenate([h32[sl, t], np.zeros(pad, np.float32)])
            coef_cols.append(_chunk_layout(hh))
        a16 = np.ascontiguousarray(
            np.concatenate(coef_cols, axis=1).astype(bfloat16)
        )
        assert a16.shape == (128, AUX16_COLS)

        vs = slice(k * V_SHARD, (k + 1) * V_SHARD)
        vpad = V_CHUNKS * 128 - V_SHARD
        xv = np.concatenate([bt32[vs], np.zeros((vpad, B), np.float32)])
        xv = xv.reshape(V_CHUNKS, 128, B).transpose(1, 0, 2).reshape(128, V_CHUNKS * B)
        wv = np.concatenate([w[vs], np.zeros(vpad, np.float32)])
        wv = np.ascontiguousarray(wv.reshape(V_CHUNKS, 128).T)
        a32 = np.ascontiguousarray(
            np.concatenate([xv, wv], axis=1, dtype=np.float32)
        )
        assert a32.shape == (128, AUX32_COLS)

        in_maps.append({"xt": bt, "idx": idx_arr, "aux16": a16, "aux32": a32})

    return in_maps, const0


# ---------------------------------------------------------------- entry
def run(inputs: dict, trace: bool = False):
    from concourse import bass_utils

    in_maps, const0 = prepare_inputs(
        inputs["x"], inputs["all_vars"], inputs["all_factors"]
    )
    nc, out_name = get_program()
    res = bass_utils.run_bass_kernel_spmd(
        nc, in_maps, core_ids=list(range(N_CORES)), trace=trace
    )
    rows = (0, 32, 64) if PSUM_OUT else (0,)
    partials = np.stack(
        [
            np.asarray(r[out_name]).reshape(-1, B)[list(rows)].sum(axis=0)
            for r in res.results
        ]
    )
    ll = partials.astype(np.float64).sum(axis=0) + const0
    return ll.astype(np.float32), res


def kernel(x, binary_combinations, all_vars, all_factors):
    out, _ = run(
        {"x": x, "all_vars": all_vars, "all_factors": all_factors}
    )
    return out


# revision 23
# speedup vs baseline: 1.2550x; 1.0306x over previous
"""Trainium2 Bass kernel for nn_BinaryMNModel (binary Markov-network clique scoring).

Math: for each batch row b,
    ll[b] = sum_c sum_j f[c,j] * prod_s ( bc[j,s] ? x[b,vars[c,s]] : 1-x[b,vars[c,s]] )

Centered multilinear basis: with b_s = x[b,vars[c,s]] - 0.5 in [-0.5, 0.5],
    score[c,b] = h0[c] + sum_s h_s[c] b_s + sum_{s<t} h_st[c] b_s b_t
               + h_012[c] b0 b1 b2
(h = an 8x8 +-0.5^k transform of the 8 factor entries, exact on host in f64).
Centering shrinks |b| and the products, so bf16 rounding error stays ~1e-3
of the final sum.  Summing over cliques:
  - h0 becomes one host-side scalar,
  - linear terms fold into a V-length weight vector w (host scatter-add),
    so sum_c(linear) = b @ w  (on-device f32 matvec, V-sharded),
  - only the 4 quad/cubic monomials need the gathered values (bf16).

Sharding: cliques are sharded across the 8 cores (2500 each); the b@w matvec
is V-sharded.  Each core returns a partial [256] vector; host sums them.

Per core on device:
  - dma_gather (GPSIMD SWDGE) pulls bf16 rows of bt=(x^T-0.5) [V, B] from
    DRAM, one merged gather per chunk-group covering all 3 clique slots
    (fewer gpsimd instructions -> no DMASW sem-lane-reuse drains, and the
    4 SWDGE queues = 4 Q7 core pairs overlap descriptor generation, which
    at ~9 ns/row is the pacing resource).
  - DVE computes the 4 products p01, p02, p12, p012=p01*a2 per group in
    bf16 (2x_1P mode: 2 elem/cycle).
  - PE reduces each weighted monomial over cliques via bf16 matmuls with
    the h column as stationary [128,1], in the PE's 128x32 column-groups
    0/32/64 (col-group 3 is a known TRN2 HW bug), accumulating into psum
    rows 0/32/64; the f32 b@w matvec rides the row-0 chain.  The three
    rows are summed on DVE at the end.
  - idx is loaded in two slices so the first gathers start as soon as the
    first ~37KB lands instead of waiting for the full index load.
"""

import os

import numpy as np

# ---------------------------------------------------------------- constants
B = 256
V = 5000
C = 20000
S = 3
NCOMB = 8
N_CORES = 8

C_SHARD = C // N_CORES          # 2500 cliques per core
CHUNKS = 20                     # 2560 = 20 * 128
C_PAD = CHUNKS * 128            # padded cliques per core

# group sizes (in 128-clique chunks): ramped so the first products start
# early while later groups are large enough to keep all 4 queues busy
GROUP_CHUNKS = [int(t) for t in os.environ.get("K_GROUPS", "2,2,2,2,2,2,2,2,1,1,1,1").split(",")]
assert sum(GROUP_CHUNKS) == CHUNKS
N_GROUPS = len(GROUP_CHUNKS)
# leading groups whose idx is loaded in the first (small) idx DMA slice
IDX_SPLIT = int(os.environ.get("K_IDX_SPLIT", "4"))

V_SHARD = V // N_CORES          # 625
V_CHUNKS = 5                    # padded to 640 = 5 * 128

IDX_COLS_TOTAL = 3 * C_PAD // 16   # 480

# aux16 layout (bf16): [coef 4*CHUNKS | xv V_CHUNKS*B | w_hi V_CHUNKS | w_lo V_CHUNKS]
XV_OFF = 4 * CHUNKS
WVH_OFF = XV_OFF + V_CHUNKS * B
WVL_OFF = WVH_OFF + V_CHUNKS
AUX16_COLS = WVL_OFF + V_CHUNKS

N_QUEUES = int(os.environ.get("K_NQ", "4"))
MERGE_SLOTS = os.environ.get("K_MERGE", "1") == "1"   # one gather per group (vs per slot)
SCALAR_DMA = os.environ.get("K_SCALAR_DMA", "1") == "1"  # aux32 on ACT HWDGE ring
SINGLE_PACKET = os.environ.get("K_SP", "1") == "1"
DUMMY_GATHER = os.environ.get("K_DUMMY", "0") == "1"  # warm-up gather before the real ones
PSUM_OUT = os.environ.get("K_PSUM_OUT", "0") == "1"   # DMA psum rows; host sums

_PROGRAM = None  # compiled program cache: (nc, out_name)


def _group_offsets():
    c_off = [0]
    for gc in GROUP_CHUNKS:
        c_off.append(c_off[-1] + gc)
    return c_off


def _build_program():
    import concourse.bass as bass
    import concourse.mybir as mybir
    from concourse import bacc, tile

    f32 = mybir.dt.float32
    bf16 = mybir.dt.bfloat16
    i16 = mybir.dt.int16
    MULT = mybir.AluOpType.mult

    nc = bacc.Bacc(
        "TRN2",
        target_bir_lowering=False,
        debug=False,
        enable_asserts=False,
        num_devices=N_CORES,
        num_swdge_queues=max(N_QUEUES, 1),
    )

    xt_d = nc.dram_tensor("xt", [V, B], bf16, kind="ExternalInput")
    idx_d = nc.dram_tensor("idx", [128, IDX_COLS_TOTAL], i16, kind="ExternalInput")
    a16_d = nc.dram_tensor("aux16", [128, AUX16_COLS], bf16, kind="ExternalInput")
    out_rows = 65 if PSUM_OUT else 1
    out_d = nc.dram_tensor("out", [out_rows, B], f32, kind="ExternalOutput")

    c_off = _group_offsets()
    # idx column offset of each group block (each group: 3*gc*8 cols)
    idx_off = [0]
    for gc in GROUP_CHUNKS:
        idx_off.append(idx_off[-1] + 3 * gc * 8)
    split_col = idx_off[min(IDX_SPLIT, N_GROUPS)]

    with tile.TileContext(nc) as tc:
        with (
            tc.tile_pool(name="persist", bufs=1) as pp,
            tc.tile_pool(name="prod", bufs=4) as prodp,
            tc.tile_pool(name="ps", bufs=1, space="PSUM") as psp,
        ):
            idx_a = pp.tile([128, max(split_col, 8)], i16, tag="idxa")
            idx_b = pp.tile(
                [128, max(IDX_COLS_TOTAL - split_col, 8)], i16, tag="idxb"
            )

            def idx_slice(lo, hi):
                if hi <= split_col:
                    return idx_a[:, lo:hi]
                assert lo >= split_col
                return idx_b[:, lo - split_col : hi - split_col]
            a16_t = pp.tile([128, AUX16_COLS], bf16, tag="aux16")
            a_all = pp.tile([128, 3 * CHUNKS, B], bf16, tag="a_all", name="a_all")
            out_sb = pp.tile([1, B], f32, tag="out_sb")
            tmp_s = pp.tile([1, B], f32, tag="tmp_s")
            tmp_t = pp.tile([1, B], f32, tag="tmp_t")
            psum_t = psp.tile([128, B], f32, tag="psum")

            if os.environ.get("K_LOADLIB", "0") == "1":
                from concourse import library_config as _lc

                nc.gpsimd.load_library(_lc.mlp)
            if DUMMY_GATHER:
                # tiny warm-up gather: triggers the Q7 ucode load/warm path
                # before the real idx data even arrives (reads row 0 of xt)
                dum_i = pp.tile([128, 8], i16, tag="dum_i")
                dum_o = pp.tile([128, 1, B], bf16, tag="dum_o")
                nc.gpsimd.memset(dum_i[:], 0)
                nc.gpsimd.dma_gather(
                    dum_o[:], xt_d[:], dum_i[:], 128, nc.gpsimd.to_reg(128), B,
                    queue_num=0, single_packet=SINGLE_PACKET,
                )

            # input loads: first idx slice gates the first gathers; aux32
            # rides the scalar-engine HWDGE ring so it doesn't queue behind
            # the idx slices on the sync ring.
            if split_col > 0:
                nc.sync.dma_start(idx_a[:], idx_d[:, :split_col])
            if split_col < IDX_COLS_TOTAL:
                nc.sync.dma_start(idx_b[:], idx_d[:, split_col:])
            aux_eng = nc.scalar if SCALAR_DMA else nc.sync
            aux_eng.dma_start(a16_t[:], a16_d[:])

            # all gathers first: they are gpsimd's only work and pace the
            # kernel.  One merged gather per group (3 slots concatenated);
            # queue assignment must be pure round-robin: Tile's 8 DMASW sem
            # lanes are assigned round-robin per call and each lane is locked
            # to one SWDGE queue.  num_idxs registers are hoisted: a MOVE on
            # the Pool SEQ costs ~0.5us, so emit one per distinct size, not
            # one per call.
            size_regs = {}
            sizes = set()
            for gc in GROUP_CHUNKS:
                sizes.add(3 * gc * 128 if MERGE_SLOTS else gc * 128)
            for n in sorted(sizes):
                size_regs[n] = nc.gpsimd.to_reg(n)
            gi = 0
            for g, gc in enumerate(GROUP_CHUNKS):
                c0 = c_off[g]
                if MERGE_SLOTS:
                    n_idx = 3 * gc * 128
                    nc.gpsimd.dma_gather(
                        a_all[:, 3 * c0 : 3 * c0 + 3 * gc, :],
                        xt_d[:],
                        idx_slice(idx_off[g], idx_off[g + 1]),
                        n_idx,
                        size_regs[n_idx],
                        B,
                        queue_num=(gi + DUMMY_GATHER) % N_QUEUES,
                        single_packet=SINGLE_PACKET,
                    )
                    gi += 1
                else:
                    for s in range(S):
                        n_idx = gc * 128
                        nc.gpsimd.dma_gather(
                            a_all[:, 3 * c0 + s * gc : 3 * c0 + (s + 1) * gc, :],
                            xt_d[:],
                            idx_slice(
                                idx_off[g] + s * gc * 8,
                                idx_off[g] + (s + 1) * gc * 8,
                            ),
                            n_idx,
                            size_regs[n_idx],
                            B,
                            queue_num=(gi + DUMMY_GATHER) % N_QUEUES,
                            single_packet=SINGLE_PACKET,
                        )
                        gi += 1

            # PE accumulation: weighted clique reductions run in the PE's
            # 128x32 column-groups (col-group 3 is a known HW bug, so only
            # groups 0/32/64 are used).  m01 -> row 0, m02 -> row 32,
            # m12 -> row 64; m012 is split between rows 32 and 64.
            row_started = set()

            def mm(mono, chunk, moving_ap, last=False):
                coef_col = mono * CHUNKS + chunk
                lhs = a16_t[:, coef_col : coef_col + 1]
                if mono < 3:
                    row = 32 * mono
                else:
                    row = 32 if chunk < CHUNKS // 2 else 64
                nc.tensor.matmul(
                    psum_t[row : row + 1, :],
                    lhs,
                    moving_ap,
                    start=(row not in row_started),
                    stop=last,
                    tile_position=(0, row),
                )
                row_started.add(row)

            def matvec():
                # linear terms: b @ (w_hi + w_lo), both bf16 (error-split
                # weights) on the PE row-0 chain -- avoids the ~6us fp32
                # LOW/HIGH blob in the in-order PE queue
                for off in (WVH_OFF, WVL_OFF):
                    for j in range(V_CHUNKS):
                        nc.tensor.matmul(
                            psum_t[0:1, :],
                            a16_t[:, off + j : off + j + 1],
                            a16_t[:, XV_OFF + j * B : XV_OFF + (j + 1) * B],
                            start=(0 not in row_started),
                            stop=False,
                            tile_position=(0, 0),
                        )
                        row_started.add(0)

            for g, gc in enumerate(GROUP_CHUNKS):
                c0 = c_off[g]
                base = 3 * c0
                a0g = a_all[:, base : base + gc, :]
                a1g = a_all[:, base + gc : base + 2 * gc, :]
                a2g = a_all[:, base + 2 * gc : base + 3 * gc, :]
                p01 = prodp.tile([128, gc, B], bf16, tag="p01", name="p01")
                p02 = prodp.tile([128, gc, B], bf16, tag="p02", name="p02")
                p12 = prodp.tile([128, gc, B], bf16, tag="p12", name="p12")
                p012 = prodp.tile([128, gc, B], bf16, tag="p012", name="p012")
                nc.vector.tensor_tensor(p01[:], a0g, a1g, MULT)
                nc.vector.tensor_tensor(p02[:], a0g, a2g, MULT)
                nc.vector.tensor_tensor(p12[:], a1g, a2g, MULT)
                nc.vector.tensor_tensor(p012[:], p01[:], a2g, MULT)
                last = g == N_GROUPS - 1
                for c in range(gc):
                    ci = c0 + c
                    lc = last and c == gc - 1
                    mm(0, ci, p01[:, c, :], last=lc)   # row 0 ends with m01
                    mm(1, ci, p02[:, c, :], last=lc)   # row 32 ends with m02
                    mm(2, ci, p12[:, c, :])
                    mm(3, ci, p012[:, c, :], last=lc)  # row 64 ends with m012
                if g == min(2, N_GROUPS - 1):
                    # matvec mid-stream: aux32 has landed by now, and the
                    # fp32 LOW/HIGH pair must not open the row-0 chain
                    matvec()

            if PSUM_OUT:
                # one wide copy of psum partitions 0..64 to SBUF, then DMA;
                # the host sums rows 0/32/64 (the three accumulation chains)
                out65 = pp.tile([65, B], f32, tag="out65")
                nc.vector.tensor_copy(out65[:], psum_t[0:65, :])
                nc.sync.dma_start(out_d[:], out65[:])
            else:
                # combine the 3 chain rows (0, 32, 64) into the output
                # (DVE may read at most one PSUM operand per instruction)
                nc.vector.tensor_copy(tmp_s[:], psum_t[0:1, :])
                nc.vector.tensor_add(tmp_t[:], tmp_s[:], psum_t[32:33, :])
                nc.vector.tensor_add(out_sb[:], tmp_t[:], psum_t[64:65, :])
                nc.sync.dma_start(out_d[:], out_sb[:])

    nc.compile()
    return nc, out_d.name


def get_program():
    global _PROGRAM
    if _PROGRAM is None:
        _PROGRAM = _build_program()
    return _PROGRAM


# ---------------------------------------------------------------- host prep
def _monomial_transform_centered(all_factors: np.ndarray) -> np.ndarray:
    """h[c,T] such that score[c,b] = sum_T h[c,T] * prod_{s in T} b_s,
    with b_s = a_s - 0.5 and T a bitmask over slots (bit s = slot s)."""
    M = np.zeros((NCOMB, NCOMB), dtype=np.float64)
    for T in range(NCOMB):
        nT = bin(T).count("1")
        for j in range(NCOMB):
            prod = 0.5 ** (S - nT)
            for s in range(S):
                if (T >> s) & 1:
                    prod *= 1.0 if (j >> (S - 1 - s)) & 1 else -1.0
            M[T, j] = prod
    return all_factors.astype(np.float64) @ M.T


def _wrap_idx(idx: np.ndarray) -> np.ndarray:
    """[N] int -> [128, N//16] int16 dma_gather layout (idx i at
    partition i%16, col i//16, replicated across the 8 q7 cores)."""
    w = idx.reshape(-1, 16).T.astype(np.int16)  # [16, N//16]
    return np.tile(w, (8, 1))


def _chunk_layout(v: np.ndarray) -> np.ndarray:
    """[C_PAD] -> [128, CHUNKS]: element i at partition i%128, col i//128."""
    return np.ascontiguousarray(v.reshape(CHUNKS, 128).T)


def prepare_inputs(x, all_vars, all_factors):
    import ml_dtypes

    bfloat16 = ml_dtypes.bfloat16
    x = np.asarray(x, dtype=np.float32)
    all_vars = np.asarray(all_vars)
    all_factors = np.asarray(all_factors, dtype=np.float32)

    bt32 = np.ascontiguousarray(x.T) - np.float32(0.5)  # [V, B] f32 centered
    bt = bt32.astype(bfloat16)                          # [V, B] bf16

    h = _monomial_transform_centered(all_factors)  # [C, 8] f64
    T01, T02, T12, T012 = 0b011, 0b101, 0b110, 0b111

    const0 = float(h[:, 0].sum())
    w = np.zeros(V, dtype=np.float64)
    for s in range(S):
        np.add.at(w, all_vars[:, s], h[:, 1 << s])
    h32 = h.astype(np.float32)

    c_off = _group_offsets()
    in_maps = []
    for k in range(N_CORES):
        sl = slice(k * C_SHARD, (k + 1) * C_SHARD)
        pad = C_PAD - C_SHARD

        # padded per-core vars: [C_PAD, S]; clique ci at (p=ci%128, chunk=ci//128)
        vp = np.concatenate(
            [all_vars[sl], np.zeros((pad, S), all_vars.dtype)], axis=0
        )
        idx_blocks = []
        for g, gc in enumerate(GROUP_CHUNKS):
            c0 = c_off[g]
            # row i = s*(gc*128) + c_local*128 + p -> var of clique
            # (c0+c_local)*128 + p, slot s
            blk = np.concatenate(
                [
                    vp[c0 * 128 : (c0 + gc) * 128, s]
                    for s in range(S)
                ]
            )
            idx_blocks.append(_wrap_idx(blk))
        idx_arr = np.ascontiguousarray(np.concatenate(idx_blocks, axis=1))
        assert idx_arr.shape == (128, IDX_COLS_TOTAL)

        coef_cols = []
        for t in (T01, T02, T12, T012):
            hh = np.concatenate([h32[sl, t], np.zeros(pad, np.float32)])
            coef_cols.append(_chunk_layout(hh))
        coef_cols_arr = np.concatenate(coef_cols, axis=1).astype(bfloat16)

        vs = slice(k * V_SHARD, (k + 1) * V_SHARD)
        vpad = V_CHUNKS * 128 - V_SHARD
        xv = np.concatenate([bt32[vs], np.zeros((vpad, B), np.float32)])
        xv = xv.reshape(V_CHUNKS, 128, B).transpose(1, 0, 2).reshape(128, V_CHUNKS * B)
        wk = np.concatenate([w[vs].astype(np.float64), np.zeros(vpad)])
        w_hi = wk.astype(bfloat16)
        w_lo = (wk - w_hi.astype(np.float64)).astype(bfloat16)
        cols = [
            coef_cols_arr,
            xv.astype(bfloat16),
            np.ascontiguousarray(w_hi.reshape(V_CHUNKS, 128).T),
            np.ascontiguousarray(w_lo.reshape(V_CHUNKS, 128).T),
        ]
        a16full = np.ascontiguousarray(np.concatenate(cols, axis=1).astype(bfloat16))
        assert a16full.shape == (128, AUX16_COLS)

        in_maps.append({"xt": bt, "idx": idx_arr, "aux16": a16full})

    return in_maps, const0


# ---------------------------------------------------------------- entry
def run(inputs: dict, trace: bool = False):
    from concourse import bass_utils

    in_maps, const0 = prepare_inputs(
        inputs["x"], inputs["all_vars"], inputs["all_factors"]
    )
    nc, out_name = get_program()
    res = bass_utils.run_bass_kernel_spmd(
        nc, in_maps, core_ids=list(range(N_CORES)), trace=trace
    )
    rows = (0, 32, 64) if PSUM_OUT else (0,)
    partials = np.stack(
        [
            np.asarray(r[out_name]).reshape(-1, B)[list(rows)].sum(axis=0)
            for r in res.results
        ]
    )
    ll = partials.astype(np.float64).sum(axis=0) + const0
    return ll.astype(np.float32), res


def kernel(x, binary_combinations, all_vars, all_factors):
    out, _ = run(
        {"x": x, "all_vars": all_vars, "all_factors": all_factors}
    )
    return out
